# revision 26
# baseline (speedup 1.0000x reference)
"""Trainium2 Bass kernel for nn_ARDG (topk_masking, B=256, N=128, DX=8, DE=5).

Data-parallel over 8 NeuronCores (32 graphs each). Key algorithmic facts
(step_size == 1, node_mask all ones):
  - At most ONE masked edge (u,v) and ONE masked node u get unmasked per
    graph: the argmax-gumbel among masked positions.
  - The discriminator loop collapses: d_i = temp * (base + delta_i) where
    base = Sx/N + Se/N^2 + y@wy and delta is a per-class weight offset.
  - E_out == E except rows (u,v),(v,u) -> one_hot(pred_e); X_out == X
    except row u -> one_hot(pred_x).
So the kernel streams E through SBUF (stats + bulk copy out), finds
argmaxes with max_with_indices, computes the tiny per-graph scalar chain,
gathers logits/gumbels at the selected positions with indirect DMA, and
scatters the few fixed-up rows (OOB-gated when nothing is masked).

Engine split per graph (phase 1):
  ACT: NEG = E[...,5]*2^30 - 2^30 (the only strided E read)
  DVE: cnt partial = reduce(NEG); m = g_masked + NEG; max_with_indices(m)
  PE : 6 column-sum matmuls of the E slab (for the we-weighted sum), then
       DVE collapses them against a host-built weight pattern W2.
g_edge arrives host-premasked with -2^30 on the lower triangle+diagonal,
so no triu mask is needed on device; symmetric counting (sum E5 == 2 *
masked_e) replaces the masked count.
"""

import os
import sys

import numpy as np

for _p in ("/opt/trn_rl_repo", "/root/.axon_site/_ro/trn_rl_repo"):
    if _p not in sys.path and os.path.isdir(_p):
        sys.path.insert(0, _p)

from concourse import bacc, bass, mybir, tile  # noqa: E402
from concourse.bass_utils import run_bass_kernel_spmd  # noqa: E402
from concourse.tile import add_dep_helper  # noqa: E402

F32 = mybir.dt.float32
I32 = mybir.dt.int32
U32 = mybir.dt.uint32
ALU = mybir.AluOpType
ACTF = mybir.ActivationFunctionType
AX = mybir.AxisListType

B, N, DX, DE, DY = 256, 128, 8, 5, 4
NCORES = 8
BL = B // NCORES  # graphs per core
BIG = float(2**30)  # exact in f32; sums of up to 2^14 of them stay exact
OOB = float(2**23)  # gated-off index offset: > all bounds, idx*coef*4B < 2^31
EW = N * (DE + 1)  # 768: free width of one graph's E slab
NCHUNK = EW // N  # 6 column-sum matmul chunks per graph


# consts block column layout
def const_layout(bl):
    o = {}
    o["wxr"] = 0                      # wx tiled per (g, c): 9*bl
    o["ident"] = o["wxr"] + (DX + 1) * bl
    o["iotar"] = o["ident"] + N       # row iota 0..127
    o["iotac"] = o["iotar"] + N       # col: value = partition index
    o["wyr"] = o["iotac"] + 1         # wy broadcast: 4
    o["we6"] = o["wyr"] + DY          # we broadcast: 6
    o["w2"] = o["we6"] + (DE + 1)     # W2[p, k] = we[(128k + p) % 6]: 6
    o["w2g"] = o["w2"] + NCHUNK       # W2 tiled 4x (for grouped SCR): 24
    o["nconst"] = o["w2g"] + 4 * NCHUNK
    return o


def build_consts(we, wx, wy, bl=BL):
    o = const_layout(bl)
    c = np.zeros((128, o["nconst"]), dtype=np.float32)
    c[:, o["wxr"] : o["ident"]] = np.tile(wx, bl)[None, :]
    c[:, o["ident"] : o["iotar"]] = np.eye(N, dtype=np.float32)
    c[:, o["iotar"] : o["iotac"]] = np.arange(N, dtype=np.float32)[None, :]
    c[:, o["iotac"]] = np.arange(128, dtype=np.float32)
    c[:, o["wyr"] : o["we6"]] = wy[None, :]
    c[:, o["we6"] : o["w2"]] = we[None, :]
    p = np.arange(128)
    for k in range(NCHUNK):
        c[:, o["w2"] + k] = we[(128 * k + p) % (DE + 1)]
    c[:, o["w2g"] : o["w2g"] + 4 * NCHUNK] = np.tile(
        c[:, o["w2"] : o["w2"] + NCHUNK], (1, 4)
    )
    return c


_TRIL_NEG = None


def premask_g_edge(g_edge):
    """g_edge + (-2^30) on diagonal and lower triangle (host-side)."""
    global _TRIL_NEG
    if _TRIL_NEG is None:
        _TRIL_NEG = np.tril(np.ones((N, N), np.float32)) * np.float32(-BIG)
    return (np.asarray(g_edge, np.float32) + _TRIL_NEG[None]).astype(np.float32)


def build_nc(bl=BL, group=4):
    assert bl % group == 0
    nc = bacc.Bacc()

    E_in = nc.declare_dram_parameter("E", [bl, N, N, DE + 1], F32, isOutput=False)
    ge = nc.declare_dram_parameter("g_edge", [bl, N, N], F32, isOutput=False)
    X_in = nc.declare_dram_parameter("X", [bl, N, DX + 1], F32, isOutput=False)
    gn = nc.declare_dram_parameter("g_node", [bl, N], F32, isOutput=False)
    lx = nc.declare_dram_parameter("logits_x", [bl, N, DX], F32, isOutput=False)
    gcx = nc.declare_dram_parameter("g_cat_x", [bl, N, DX], F32, isOutput=False)
    le = nc.declare_dram_parameter("logits_e", [bl, N, N, DE], F32, isOutput=False)
    gce = nc.declare_dram_parameter("g_cat_e", [bl, N, N, DE], F32, isOutput=False)
    yy = nc.declare_dram_parameter("y", [bl, DY], F32, isOutput=False)
    OL = const_layout(bl)
    cst = nc.declare_dram_parameter("consts", [128, OL["nconst"]], F32, isOutput=False)
    E_out = nc.declare_dram_parameter("E_out", [bl, N, N, DE + 1], F32, isOutput=True)
    X_out = nc.declare_dram_parameter("X_out", [bl, N, DX + 1], F32, isOutput=True)

    with tile.TileContext(nc) as tc:
        with (
            tc.tile_pool(name="epool", bufs=6) as epool,
            tc.tile_pool(name="scratch", bufs=3) as spool,
            tc.tile_pool(name="persist", bufs=1) as pp,
            tc.tile_pool(name="small", bufs=1) as sm,
            tc.tile_pool(name="psum", bufs=1, space="PSUM") as psp,
            tc.tile_pool(name="psum6", bufs=2, space="PSUM") as psp6,
        ):
            # ---- constants / persistent state ----
            CT = pp.tile([128, OL["nconst"]], F32, tag="consts")
            nc.sync.dma_start(CT[:], cst[:])
            WXR = CT[:, OL["wxr"] : OL["ident"]]
            IDENT = CT[:, OL["ident"] : OL["iotar"]]
            W2 = CT[:, OL["w2"] : OL["w2"] + NCHUNK]
            W2G = CT[:, OL["w2g"] : OL["w2g"] + group * NCHUNK]
            WE_B = CT[0:bl, OL["we6"] : OL["we6"] + DE + 1]
            WX_B = CT[0:bl, OL["wxr"] : OL["wxr"] + DX + 1]
            IOTAC = CT[0:bl, OL["iotac"] : OL["iotac"] + 1]  # value = partition = g
            IOTA6 = CT[0:bl, OL["iotar"] : OL["iotar"] + DE + 1]  # 0..5 per row
            IOTA9 = CT[0:bl, OL["iotar"] : OL["iotar"] + DX + 1]  # 0..8 per row
            WY_B = CT[0:bl, OL["wyr"] : OL["wyr"] + DY]

            ONESC = pp.tile([128, 1], F32, tag="ones")
            nc.gpsimd.memset(ONESC[:], 1.0)
            # preload ACT Exp/Ln tables so the late softmax doesn't stall
            DUM = sm.tile([1, 1], F32, tag="dum")
            nc.scalar.activation(DUM[:], ONESC[0:1, 0:1], ACTF.Exp)
            nc.scalar.activation(DUM[:], DUM[:], ACTF.Ln)

            CNT = pp.tile([128, bl], F32, tag="cnt")  # per-row sum(NEG) partials
            SEP = pp.tile([128, bl], F32, tag="sep")  # per-row E*we chunk partials
            MX8 = pp.tile([128, 8 * bl], F32, tag="mx8")  # per-row top8 of m
            MI8 = pp.tile([128, 8 * bl], U32, tag="mi8")

            # ---- small batched inputs ----
            GT = pp.tile([128, bl * N], F32, tag="gt")  # g_edge (premasked), [i,(g j)]
            nc.gpsimd.dma_start(
                GT[:].rearrange("i (g j) -> i g j", g=bl),
                ge[:].rearrange("g i j -> i g j"),
            )
            XT = pp.tile([128, bl * (DX + 1)], F32, tag="xt")  # X, [n, (g c)]
            nc.scalar.dma_start(
                XT[:].rearrange("n (g c) -> n g c", g=bl),
                X_in[:].rearrange("g n c -> n g c"),
            )
            GN = pp.tile([128, bl], F32, tag="gn")  # g_node, [n, g]
            nc.scalar.dma_start(GN[:], gn[:].rearrange("g n -> n g"))
            Y32 = pp.tile([bl, DY], F32, tag="y32")
            nc.scalar.dma_start(Y32[:], yy[:])

            # ---- phase 2: X-side stats ----
            XT3 = XT[:].rearrange("n (g c) -> n g c", g=bl)
            X8 = XT3[:, :, DX]  # [128, bl] masked-node indicator
            SXS = sm.tile([128, bl * (DX + 1)], F32, tag="sxs")
            nc.vector.tensor_tensor(out=SXS[:], in0=XT[:], in1=WXR, op=ALU.mult)
            SXP = sm.tile([128, bl], F32, tag="sxp")
            nc.vector.tensor_reduce(
                out=SXP[:], in_=SXS[:].rearrange("n (g c) -> n g c", g=bl),
                axis=AX.X, op=ALU.add,
            )
            NEGN = sm.tile([128, bl], F32, tag="negn")
            nc.scalar.activation(NEGN[:], X8, ACTF.Copy, bias=-BIG, scale=BIG)
            MN = sm.tile([128, bl], F32, tag="mn")
            nc.vector.tensor_tensor(out=MN[:], in0=GN[:], in1=NEGN[:], op=ALU.add)
            MNT_P = psp.tile([bl, 128], F32, tag="tp")
            nc.tensor.transpose(out=MNT_P[:], in_=MN[:], identity=IDENT)
            MNT = sm.tile([bl, 128], F32, tag="mnt")
            nc.vector.tensor_copy(out=MNT[:], in_=MNT_P[:])
            NMX = sm.tile([bl, 8], F32, tag="nmx")
            NIX = sm.tile([bl, 8], U32, tag="nix")
            nc.vector.max_with_indices(out_max=NMX[:], out_indices=NIX[:], in_=MNT[:])
            UNF = sm.tile([bl, 1], F32, tag="unf")
            nc.vector.tensor_copy(out=UNF[:], in_=NIX[:, 0:1])
            nc.vector.tensor_scalar(
                out=UNF[:], in0=UNF[:], scalar1=float(N - 1), scalar2=0.0,
                op0=ALU.min, op1=ALU.max,
            )

            P4 = psp.tile([bl, 4], F32, tag="p4")
            P_CNTN, P_CNTE, P_SE, P_SX = (P4[:, i : i + 1] for i in range(4))
            nc.tensor.matmul(out=P_CNTN, lhsT=X8, rhs=ONESC[:], start=True, stop=True)
            nc.tensor.matmul(out=P_SX, lhsT=SXP[:], rhs=ONESC[:], start=True, stop=True)
            CNTN = sm.tile([bl, 1], F32, tag="cntn")
            nc.vector.tensor_copy(out=CNTN[:], in_=P_CNTN)
            SX = sm.tile([bl, 1], F32, tag="sx")
            nc.vector.tensor_copy(out=SX[:], in_=P_SX)
            GATE_N = sm.tile([bl, 1], F32, tag="gate_n")
            nc.vector.tensor_scalar(
                out=GATE_N[:], in0=CNTN[:], scalar1=1.0, scalar2=None, op0=ALU.min
            )
            OFF_N = sm.tile([bl, 1], F32, tag="off_n")
            nc.vector.tensor_scalar(
                out=OFF_N[:], in0=GATE_N[:], scalar1=-OOB, scalar2=OOB,
                op0=ALU.mult, op1=ALU.add,
            )
            GNF = sm.tile([bl, 1], F32, tag="gnf")
            nc.vector.scalar_tensor_tensor(
                out=GNF[:], in0=IOTAC, scalar=float(N), in1=OFF_N[:],
                op0=ALU.mult, op1=ALU.add,
            )
            FN = sm.tile([bl, 1], F32, tag="fn")
            nc.vector.tensor_tensor(out=FN[:], in0=GNF[:], in1=UNF[:], op=ALU.add)
            FNI = sm.tile([bl, 1], I32, tag="fni")
            nc.vector.tensor_copy(out=FNI[:], in_=FN[:])

            LXG = sm.tile([bl, DX], F32, tag="lxg")
            nc.gpsimd.memset(LXG[:], 0.0)
            GXG = sm.tile([bl, DX], F32, tag="gxg")
            nc.gpsimd.memset(GXG[:], 0.0)
            nc.gpsimd.indirect_dma_start(
                out=LXG[:], out_offset=None,
                in_=lx[:].rearrange("g n c -> (g n) c"),
                in_offset=bass.IndirectOffsetOnAxis(ap=FNI[:, 0:1], axis=0),
                bounds_check=bl * N - 1, oob_is_err=False,
            )
            nc.gpsimd.indirect_dma_start(
                out=GXG[:], out_offset=None,
                in_=gcx[:].rearrange("g n c -> (g n) c"),
                in_offset=bass.IndirectOffsetOnAxis(ap=FNI[:, 0:1], axis=0),
                bounds_check=bl * N - 1, oob_is_err=False,
            )

            # ---- phase 1: stream E, per-graph row stats ----
            bulk_writes = []
            GT3 = GT[:].rearrange("i (g j) -> i g j", g=bl)
            for g0 in range(0, bl, group):
                T = epool.tile([128, group * EW], F32, tag="etile")
                T3 = T[:].rearrange("p (g f) -> p g f", g=group)
                eng_in = [nc.sync, nc.scalar, nc.sync, nc.scalar,
                          nc.sync, nc.sync, nc.scalar, nc.sync][(g0 // group) % 8]
                eng_in.dma_start(
                    T3, E_in[g0 : g0 + group].rearrange("g i j c -> i g (j c)")
                )
                eng_out = [nc.gpsimd, nc.gpsimd, nc.gpsimd, nc.gpsimd,
                           nc.gpsimd, nc.sync, nc.scalar, nc.sync][(g0 // group) % 8]
                w = eng_out.dma_start(
                    E_out[g0 : g0 + group].rearrange("g i j c -> i g (j c)"), T3
                )
                bulk_writes.append(w)
                # grouped stats: one ACT + three DVE ops per 4-graph group
                E5G = T3.rearrange("p g (j c) -> p g j c", c=DE + 1)[:, :, :, DE]
                NEG4 = spool.tile([128, group * N], F32, tag="neg4")
                nc.scalar.activation(
                    NEG4[:].rearrange("p (g j) -> p g j", g=group), E5G,
                    ACTF.Copy, bias=-BIG, scale=BIG,
                )
                nc.vector.tensor_reduce(
                    out=CNT[:, g0 : g0 + group],
                    in_=NEG4[:].rearrange("p (g j) -> p g j", g=group),
                    axis=AX.X, op=ALU.add,
                )
                M4 = spool.tile([128, group * N], F32, tag="m4")
                nc.vector.tensor_tensor(
                    out=M4[:], in0=GT3[:, g0 : g0 + group].rearrange("p g j -> p (g j)"),
                    in1=NEG4[:], op=ALU.add,
                )
                PS6G = psp6.tile([128, group * NCHUNK], F32, tag="ps6g")
                for k in range(group):
                    g = g0 + k
                    nc.vector.max_with_indices(
                        out_max=MX8[:, 8 * g : 8 * g + 8],
                        out_indices=MI8[:, 8 * g : 8 * g + 8],
                        in_=M4[:, k * N : (k + 1) * N],
                    )
                    for c in range(NCHUNK):
                        nc.tensor.matmul(
                            out=PS6G[:, k * NCHUNK + c : k * NCHUNK + c + 1],
                            lhsT=T3[:, k][:, c * N : (c + 1) * N],
                            rhs=ONESC[:],
                            start=True,
                            stop=True,
                        )
                SCR6 = spool.tile([128, group * NCHUNK], F32, tag="scr6")
                nc.vector.tensor_tensor(
                    out=SCR6[:], in0=PS6G[:], in1=W2G, op=ALU.mult
                )
                nc.vector.tensor_reduce(
                    out=SEP[:, g0 : g0 + group],
                    in_=SCR6[:].rearrange("p (g k) -> p g k", g=group),
                    axis=AX.X, op=ALU.add,
                )

            # X_out bulk write (from XT, unmodified)
            xw = nc.gpsimd.dma_start(
                X_out[:].rearrange("g n c -> n g c"),
                XT[:].rearrange("n (g c) -> n g c", g=bl),
            )

            # ---- phase 3: edge argmax (u, v) ----
            RMAX = sm.tile([128, bl], F32, tag="rmax")
            nc.vector.tensor_copy(
                out=RMAX[:], in_=MX8[:].rearrange("p (g e) -> p g e", e=8)[:, :, 0]
            )
            RIDX = sm.tile([128, bl], F32, tag="ridx")
            nc.vector.tensor_copy(
                out=RIDX[:], in_=MI8[:].rearrange("p (g e) -> p g e", e=8)[:, :, 0]
            )
            RMAXT_P = psp.tile([bl, 128], F32, tag="tp")
            nc.tensor.transpose(out=RMAXT_P[:], in_=RMAX[:], identity=IDENT)
            RMAXT = sm.tile([bl, 128], F32, tag="rmaxt")
            nc.vector.tensor_copy(out=RMAXT[:], in_=RMAXT_P[:])
            RIDXT_P = psp.tile([bl, 128], F32, tag="tp")
            nc.tensor.transpose(out=RIDXT_P[:], in_=RIDX[:], identity=IDENT)
            RIDXT = sm.tile([bl, 128], F32, tag="ridxt")
            nc.vector.tensor_copy(out=RIDXT[:], in_=RIDXT_P[:])
            EMX = sm.tile([bl, 8], F32, tag="emx")
            EIX = sm.tile([bl, 8], U32, tag="eix")
            nc.vector.max_with_indices(out_max=EMX[:], out_indices=EIX[:], in_=RMAXT[:])
            UEF = sm.tile([bl, 1], F32, tag="uef")
            nc.vector.tensor_copy(out=UEF[:], in_=EIX[:, 0:1])
            nc.vector.tensor_scalar(
                out=UEF[:], in0=UEF[:], scalar1=float(N - 1), scalar2=0.0,
                op0=ALU.min, op1=ALU.max,
            )
            EQM = sm.tile([bl, 128], F32, tag="eqm")
            nc.vector.tensor_scalar(
                out=EQM[:], in0=RMAXT[:], scalar1=EMX[:, 0:1], scalar2=None,
                op0=ALU.is_equal,
            )
            VSCR = sm.tile([bl, 128], F32, tag="vscr")
            VF = sm.tile([bl, 1], F32, tag="vf")
            nc.vector.tensor_tensor(out=VSCR[:], in0=EQM[:], in1=RIDXT[:], op=ALU.mult)
            nc.vector.tensor_reduce(out=VF[:], in_=VSCR[:], axis=AX.X, op=ALU.add)
            nc.vector.tensor_scalar(
                out=VF[:], in0=VF[:], scalar1=float(N - 1), scalar2=0.0,
                op0=ALU.min, op1=ALU.max,
            )

            # ---- phase 3: per-graph totals via matmul (partition = graph) ----
            nc.tensor.matmul(out=P_CNTE, lhsT=CNT[:], rhs=ONESC[:], start=True, stop=True)
            nc.tensor.matmul(out=P_SE, lhsT=SEP[:], rhs=ONESC[:], start=True, stop=True)
            # masked_e = N^2/2 + sum(NEG)/2^31  (sum E5 == 2*masked_e, diag 0)
            ME = sm.tile([bl, 1], F32, tag="me")
            nc.vector.tensor_scalar(
                out=ME[:], in0=P_CNTE, scalar1=float(2.0 ** -31),
                scalar2=float(N * N / 2), op0=ALU.mult, op1=ALU.add,
            )
            SE = sm.tile([bl, 1], F32, tag="se")
            nc.vector.tensor_copy(out=SE[:], in_=P_SE)

            # Sy
            YS = sm.tile([bl, DY], F32, tag="ys")
            SY = sm.tile([bl, 1], F32, tag="sy")
            nc.vector.tensor_tensor(out=YS[:], in0=Y32[:], in1=WY_B, op=ALU.mult)
            nc.vector.tensor_reduce(out=SY[:], in_=YS[:], axis=AX.X, op=ALU.add)

            # temp = 0.5 - (cntn + masked_e)*(0.5/8256); base = SX/N + SE/N^2 + SY
            NUMEL = N + N * (N - 1) / 2.0
            T1 = sm.tile([bl, 1], F32, tag="t1")
            nc.vector.tensor_tensor(out=T1[:], in0=CNTN[:], in1=ME[:], op=ALU.add)
            TEMP = sm.tile([bl, 1], F32, tag="temp")
            nc.vector.tensor_scalar(
                out=TEMP[:], in0=T1[:], scalar1=-0.5 / NUMEL, scalar2=0.5,
                op0=ALU.mult, op1=ALU.add,
            )
            B1 = sm.tile([bl, 1], F32, tag="b1")
            nc.vector.tensor_scalar(
                out=B1[:], in0=SX[:], scalar1=1.0 / N, scalar2=None, op0=ALU.mult
            )
            B2 = sm.tile([bl, 1], F32, tag="b2")
            nc.vector.scalar_tensor_tensor(
                out=B2[:], in0=SE[:], scalar=1.0 / (N * N), in1=SY[:],
                op0=ALU.mult, op1=ALU.add,
            )
            BASE = sm.tile([bl, 1], F32, tag="base")
            nc.vector.tensor_tensor(out=BASE[:], in0=B1[:], in1=B2[:], op=ALU.add)

            # ---- gates and flat indices ----
            GATE_E = sm.tile([bl, 1], F32, tag="gate_e")
            nc.vector.tensor_scalar(
                out=GATE_E[:], in0=ME[:], scalar1=1.0, scalar2=None, op0=ALU.min
            )
            OFF_E = sm.tile([bl, 1], F32, tag="off_e")  # (1-gate)*OOB
            nc.vector.tensor_scalar(
                out=OFF_E[:], in0=GATE_E[:], scalar1=-OOB, scalar2=OOB,
                op0=ALU.mult, op1=ALU.add,
            )
            GOF = sm.tile([bl, 1], F32, tag="gof")  # g*N^2 + (1-gate)*OOB
            nc.vector.scalar_tensor_tensor(
                out=GOF[:], in0=IOTAC, scalar=float(N * N), in1=OFF_E[:],
                op0=ALU.mult, op1=ALU.add,
            )
            FE = sm.tile([bl, 1], F32, tag="fe")  # g*N^2 + u*N + v (+oob)
            nc.vector.scalar_tensor_tensor(
                out=FE[:], in0=UEF[:], scalar=float(N), in1=GOF[:],
                op0=ALU.mult, op1=ALU.add,
            )
            nc.vector.tensor_tensor(out=FE[:], in0=FE[:], in1=VF[:], op=ALU.add)
            FS = sm.tile([bl, 1], F32, tag="fs")  # g*N^2 + v*N + u (+oob)
            nc.vector.scalar_tensor_tensor(
                out=FS[:], in0=VF[:], scalar=float(N), in1=GOF[:],
                op0=ALU.mult, op1=ALU.add,
            )
            nc.vector.tensor_tensor(out=FS[:], in0=FS[:], in1=UEF[:], op=ALU.add)
            FEI = sm.tile([bl, 1], I32, tag="fei")
            nc.vector.tensor_copy(out=FEI[:], in_=FE[:])
            FSI = sm.tile([bl, 1], I32, tag="fsi")
            nc.vector.tensor_copy(out=FSI[:], in_=FS[:])

            # ---- gathers at the selected positions ----
            LEG = sm.tile([bl, DE], F32, tag="leg")
            nc.gpsimd.memset(LEG[:], 0.0)
            GEG = sm.tile([bl, DE], F32, tag="geg")
            nc.gpsimd.memset(GEG[:], 0.0)
            nc.gpsimd.indirect_dma_start(
                out=LEG[:], out_offset=None,
                in_=le[:].rearrange("g u v c -> (g u v) c"),
                in_offset=bass.IndirectOffsetOnAxis(ap=FEI[:, 0:1], axis=0),
                bounds_check=bl * N * N - 1, oob_is_err=False,
            )
            nc.gpsimd.indirect_dma_start(
                out=GEG[:], out_offset=None,
                in_=gce[:].rearrange("g u v c -> (g u v) c"),
                in_offset=bass.IndirectOffsetOnAxis(ap=FEI[:, 0:1], axis=0),
                bounds_check=bl * N * N - 1, oob_is_err=False,
            )

            def softmax_scores(tag, LOG, GCAT, W_ROW, nclass, dwscale):
                # d_i = temp*(base + dwscale*(w_i - w_last)); returns scores
                DD = sm.tile([bl, nclass], F32, tag=tag + "_dd")
                nc.vector.tensor_scalar(
                    out=DD[:], in0=W_ROW[:, 0:nclass],
                    scalar1=W_ROW[:, nclass : nclass + 1], scalar2=dwscale,
                    op0=ALU.subtract, op1=ALU.mult,
                )
                nc.vector.tensor_scalar(
                    out=DD[:], in0=DD[:], scalar1=BASE[:, 0:1], scalar2=TEMP[:, 0:1],
                    op0=ALU.add, op1=ALU.mult,
                )
                ED = sm.tile([bl, nclass], F32, tag=tag + "_ed")
                nc.scalar.activation(ED[:], DD[:], ACTF.Exp)
                MXL = sm.tile([bl, 1], F32, tag=tag + "_mxl")
                nc.vector.tensor_reduce(out=MXL[:], in_=LOG[:], axis=AX.X, op=ALU.max)
                NMXL = sm.tile([bl, 1], F32, tag=tag + "_nmxl")
                nc.vector.tensor_scalar(
                    out=NMXL[:], in0=MXL[:], scalar1=-1.0, scalar2=None, op0=ALU.mult
                )
                EXL = sm.tile([bl, nclass], F32, tag=tag + "_exl")
                nc.scalar.activation(EXL[:], LOG[:], ACTF.Exp, bias=NMXL[:, 0:1])
                SML = sm.tile([bl, 1], F32, tag=tag + "_sml")
                nc.vector.tensor_reduce(out=SML[:], in_=EXL[:], axis=AX.X, op=ALU.add)
                RSL = sm.tile([bl, 1], F32, tag=tag + "_rsl")
                nc.vector.reciprocal(out=RSL[:], in_=SML[:])
                P = sm.tile([bl, nclass], F32, tag=tag + "_p")
                nc.vector.tensor_scalar(
                    out=P[:], in0=EXL[:], scalar1=RSL[:, 0:1], scalar2=None,
                    op0=ALU.mult,
                )
                GP = sm.tile([bl, nclass], F32, tag=tag + "_gp")
                nc.vector.tensor_tensor(out=GP[:], in0=P[:], in1=ED[:], op=ALU.mult)
                nc.vector.tensor_scalar(
                    out=GP[:], in0=GP[:], scalar1=1e-30, scalar2=None, op0=ALU.add
                )
                LGP = sm.tile([bl, nclass], F32, tag=tag + "_lgp")
                nc.scalar.activation(LGP[:], GP[:], ACTF.Ln)
                SCO = sm.tile([bl, nclass], F32, tag=tag + "_sco")
                nc.vector.tensor_tensor(out=SCO[:], in0=LGP[:], in1=GCAT[:], op=ALU.add)
                return SCO

            # edge prediction
            SCOE = softmax_scores("e", LEG, GEG, WE_B, DE, 2.0 / (N * N))
            SC8E = sm.tile([bl, 8], F32, tag="sc8e")
            nc.gpsimd.memset(SC8E[:], -1e30)
            nc.vector.tensor_copy(out=SC8E[:, 0:DE], in_=SCOE[:])
            PEX = sm.tile([bl, 8], F32, tag="pex")
            PEI = sm.tile([bl, 8], U32, tag="pei")
            nc.vector.max_with_indices(out_max=PEX[:], out_indices=PEI[:], in_=SC8E[:])
            PEF = sm.tile([bl, 1], F32, tag="pef")
            nc.vector.tensor_copy(out=PEF[:], in_=PEI[:, 0:1])
            OH6 = sm.tile([bl, DE + 1], F32, tag="oh6")
            nc.vector.tensor_scalar(
                out=OH6[:], in0=IOTA6, scalar1=PEF[:, 0:1], scalar2=None,
                op0=ALU.is_equal,
            )

            # node prediction
            SCOX = softmax_scores("x", LXG, GXG, WX_B, DX, 1.0 / N)
            PXX = sm.tile([bl, 8], F32, tag="pxx")
            PXI = sm.tile([bl, 8], U32, tag="pxi")
            nc.vector.max_with_indices(out_max=PXX[:], out_indices=PXI[:], in_=SCOX[:])
            PXF = sm.tile([bl, 1], F32, tag="pxf")
            nc.vector.tensor_copy(out=PXF[:], in_=PXI[:, 0:1])
            OH9 = sm.tile([bl, DX + 1], F32, tag="oh9")
            nc.vector.tensor_scalar(
                out=OH9[:], in0=IOTA9, scalar1=PXF[:, 0:1], scalar2=None,
                op0=ALU.is_equal,
            )

            # ---- scatters (ordered after bulk writes) ----
            s1 = nc.gpsimd.indirect_dma_start(
                out=E_out[:].rearrange("g u v c -> (g u v) c"),
                out_offset=bass.IndirectOffsetOnAxis(ap=FEI[:, 0:1], axis=0),
                in_=OH6[:], in_offset=None,
                bounds_check=bl * N * N - 1, oob_is_err=False,
            )
            s2 = nc.gpsimd.indirect_dma_start(
                out=E_out[:].rearrange("g u v c -> (g u v) c"),
                out_offset=bass.IndirectOffsetOnAxis(ap=FSI[:, 0:1], axis=0),
                in_=OH6[:], in_offset=None,
                bounds_check=bl * N * N - 1, oob_is_err=False,
            )
            s3 = nc.gpsimd.indirect_dma_start(
                out=X_out[:].rearrange("g n c -> (g n) c"),
                out_offset=bass.IndirectOffsetOnAxis(ap=FNI[:, 0:1], axis=0),
                in_=OH9[:], in_offset=None,
                bounds_check=bl * N - 1, oob_is_err=False,
            )
            for w in bulk_writes:
                add_dep_helper(s1.ins, w.ins, sync=True, reason="scatter after bulk E")
                add_dep_helper(s2.ins, w.ins, sync=True, reason="scatter after bulk E")
            add_dep_helper(s3.ins, xw.ins, sync=True, reason="scatter after bulk X")

    # Force every activation onto one table set (6: natural_log_exp_and_others,
    # which holds Copy/Identity/Exp/Ln) so only one LoadActFuncSet is emitted
    # instead of thrashing 0<->5 on every Copy/Exp/Ln transition.
    import concourse.bacc as _bacc_mod
    from concourse.hw_specs import get_activation_tables as _gat

    def _one_set(arch):
        t = _gat(arch)
        keys = list(t.keys())
        want = {ACTF.Copy, ACTF.Identity, ACTF.Exp, ACTF.Ln}
        full = [k for k in keys if want <= t[k]]
        assert full, f"no act set holds {want}; sets: {keys}"
        keep = full[0]
        return {k: (v if k == keep else set()) for k, v in t.items()}

    _orig = _bacc_mod.get_activation_tables
    _bacc_mod.get_activation_tables = _one_set
    try:
        nc.compile()
    finally:
        _bacc_mod.get_activation_tables = _orig
    return nc


_NC_CACHE = {}


def _get_nc(bl=BL, group=4):
    key = (bl, group)
    if key not in _NC_CACHE:
        _NC_CACHE[key] = build_nc(bl, group)
    return _NC_CACHE[key]


def make_in_maps(inputs, bl=BL, ncores=NCORES):
    f = lambda a: np.ascontiguousarray(np.asarray(a, dtype=np.float32))
    consts = build_consts(f(inputs["we"]), f(inputs["wx"]), f(inputs["wy"]), bl=bl)
    gem = premask_g_edge(inputs["g_edge"])
    in_maps = []
    for i in range(ncores):
        s = slice(i * bl, (i + 1) * bl)
        in_maps.append(
            {
                "E": f(inputs["E"][s]),
                "g_edge": np.ascontiguousarray(gem[s]),
                "X": f(inputs["X"][s]),
                "g_node": f(inputs["g_node"][s]),
                "logits_x": f(inputs["logits_x"][s]),
                "g_cat_x": f(inputs["g_cat_x"][s]),
                "logits_e": f(inputs["logits_e"][s]),
                "g_cat_e": f(inputs["g_cat_e"][s]),
                "y": f(inputs["y"][s]),
                "consts": consts,
            }
        )
    return in_maps


def kernel(**inputs):
    nc = _get_nc()
    in_maps = make_in_maps(inputs)
    res = run_bass_kernel_spmd(nc, in_maps, core_ids=list(range(NCORES)))
    X_out = np.concatenate([res.results[i]["X_out"] for i in range(NCORES)], axis=0)
    E_out = np.concatenate([res.results[i]["E_out"] for i in range(NCORES)], axis=0)
    return X_out, E_out


# revision 31
# speedup vs baseline: 1.0508x; 1.0508x over previous
"""Trainium2 Bass kernel for nn_ARDG (topk_masking, B=256, N=128, DX=8, DE=5).

Data-parallel over 8 NeuronCores (32 graphs each). Key algorithmic facts
(step_size == 1, node_mask all ones):
  - At most ONE masked edge (u,v) and ONE masked node u get unmasked per
    graph: the argmax-gumbel among masked positions.
  - The discriminator loop collapses: d_i = temp * (base + delta_i) where
    base = Sx/N + Se/N^2 + y@wy and delta is a per-class weight offset.
  - E_out == E except rows (u,v),(v,u) -> one_hot(pred_e); X_out == X
    except row u -> one_hot(pred_x).
So the kernel streams E through SBUF (stats + bulk copy out), finds
argmaxes with max_with_indices, computes the tiny per-graph scalar chain,
gathers logits/gumbels at the selected positions with indirect DMA, and
scatters the few fixed-up rows (OOB-gated when nothing is masked).

Engine split per graph (phase 1):
  ACT: NEG = E[...,5]*2^30 - 2^30 (the only strided E read)
  DVE: cnt partial = reduce(NEG); m = g_masked + NEG; max_with_indices(m)
  PE : 6 column-sum matmuls of the E slab (for the we-weighted sum), then
       DVE collapses them against a host-built weight pattern W2.
g_edge arrives host-premasked with -2^30 on the lower triangle+diagonal,
so no triu mask is needed on device; symmetric counting (sum E5 == 2 *
masked_e) replaces the masked count.
"""

import os
import sys

import numpy as np

for _p in ("/opt/trn_rl_repo", "/root/.axon_site/_ro/trn_rl_repo"):
    if _p not in sys.path and os.path.isdir(_p):
        sys.path.insert(0, _p)

from concourse import bacc, bass, mybir, tile  # noqa: E402
from concourse.bass_utils import run_bass_kernel_spmd  # noqa: E402
from concourse.tile import add_dep_helper  # noqa: E402

F32 = mybir.dt.float32
I32 = mybir.dt.int32
U32 = mybir.dt.uint32
ALU = mybir.AluOpType
ACTF = mybir.ActivationFunctionType
AX = mybir.AxisListType

B, N, DX, DE, DY = 256, 128, 8, 5, 4
NCORES = 8
BL = B // NCORES  # graphs per core
BIG = float(2**30)  # exact in f32; sums of up to 2^14 of them stay exact
OOB = float(2**23)  # gated-off index offset: > all bounds, idx*coef*4B < 2^31
EW = N * (DE + 1)  # 768: free width of one graph's E slab
NCHUNK = EW // N  # 6 column-sum matmul chunks per graph


# consts block column layout
def const_layout(bl):
    o = {}
    o["wxr"] = 0                      # wx tiled per (g, c): 9*bl
    o["ident"] = o["wxr"] + (DX + 1) * bl
    o["iotar"] = o["ident"] + N       # row iota 0..127
    o["iotac"] = o["iotar"] + N       # col: value = partition index
    o["wyr"] = o["iotac"] + 1         # wy broadcast: 4
    o["we6"] = o["wyr"] + DY          # we broadcast: 6
    o["w2"] = o["we6"] + (DE + 1)     # W2[p, k] = we[(128k + p) % 6]: 6
    o["w2g"] = o["w2"] + NCHUNK       # W2 tiled 4x (for grouped SCR): 24
    o["nconst"] = o["w2g"] + 4 * NCHUNK
    return o


def build_consts(we, wx, wy, bl=BL):
    o = const_layout(bl)
    c = np.zeros((128, o["nconst"]), dtype=np.float32)
    c[:, o["wxr"] : o["ident"]] = np.tile(wx, bl)[None, :]
    c[:, o["ident"] : o["iotar"]] = np.eye(N, dtype=np.float32)
    c[:, o["iotar"] : o["iotac"]] = np.arange(N, dtype=np.float32)[None, :]
    c[:, o["iotac"]] = np.arange(128, dtype=np.float32)
    c[:, o["wyr"] : o["we6"]] = wy[None, :]
    c[:, o["we6"] : o["w2"]] = we[None, :]
    p = np.arange(128)
    for k in range(NCHUNK):
        c[:, o["w2"] + k] = we[(128 * k + p) % (DE + 1)]
    c[:, o["w2g"] : o["w2g"] + 4 * NCHUNK] = np.tile(
        c[:, o["w2"] : o["w2"] + NCHUNK], (1, 4)
    )
    return c


_TRIL_NEG = None


def premask_g_edge(g_edge):
    """g_edge + (-2^30) on diagonal and lower triangle (host-side)."""
    global _TRIL_NEG
    if _TRIL_NEG is None:
        _TRIL_NEG = np.tril(np.ones((N, N), np.float32)) * np.float32(-BIG)
    return (np.asarray(g_edge, np.float32) + _TRIL_NEG[None]).astype(np.float32)


def build_nc(bl=BL, group=4):
    assert bl % group == 0
    nc = bacc.Bacc()

    E_in = nc.declare_dram_parameter("E", [bl, N, N, DE + 1], F32, isOutput=False)
    ge = nc.declare_dram_parameter("g_edge", [bl, N, N], F32, isOutput=False)
    X_in = nc.declare_dram_parameter("X", [bl, N, DX + 1], F32, isOutput=False)
    gn = nc.declare_dram_parameter("g_node", [bl, N], F32, isOutput=False)
    lx = nc.declare_dram_parameter("logits_x", [bl, N, DX], F32, isOutput=False)
    gcx = nc.declare_dram_parameter("g_cat_x", [bl, N, DX], F32, isOutput=False)
    le = nc.declare_dram_parameter("logits_e", [bl, N, N, DE], F32, isOutput=False)
    gce = nc.declare_dram_parameter("g_cat_e", [bl, N, N, DE], F32, isOutput=False)
    yy = nc.declare_dram_parameter("y", [bl, DY], F32, isOutput=False)
    OL = const_layout(bl)
    cst = nc.declare_dram_parameter("consts", [128, OL["nconst"]], F32, isOutput=False)
    E_out = nc.declare_dram_parameter("E_out", [bl, N, N, DE + 1], F32, isOutput=True)
    X_out = nc.declare_dram_parameter("X_out", [bl, N, DX + 1], F32, isOutput=True)

    with tile.TileContext(nc) as tc:
        with (
            tc.tile_pool(name="epool", bufs=6) as epool,
            tc.tile_pool(name="scratch", bufs=3) as spool,
            tc.tile_pool(name="persist", bufs=1) as pp,
            tc.tile_pool(name="small", bufs=1) as sm,
            tc.tile_pool(name="psum", bufs=1, space="PSUM") as psp,
            tc.tile_pool(name="psum6", bufs=2, space="PSUM") as psp6,
        ):
            # ---- constants / persistent state ----
            CT = pp.tile([128, OL["nconst"]], F32, tag="consts")
            nc.sync.dma_start(CT[:], cst[:])
            WXR = CT[:, OL["wxr"] : OL["ident"]]
            IDENT = CT[:, OL["ident"] : OL["iotar"]]
            W2 = CT[:, OL["w2"] : OL["w2"] + NCHUNK]
            W2G = CT[:, OL["w2g"] : OL["w2g"] + group * NCHUNK]
            WE_B = CT[0:bl, OL["we6"] : OL["we6"] + DE + 1]
            WX_B = CT[0:bl, OL["wxr"] : OL["wxr"] + DX + 1]
            IOTAC = CT[0:bl, OL["iotac"] : OL["iotac"] + 1]  # value = partition = g
            IOTA6 = CT[0:bl, OL["iotar"] : OL["iotar"] + DE + 1]  # 0..5 per row
            IOTA9 = CT[0:bl, OL["iotar"] : OL["iotar"] + DX + 1]  # 0..8 per row
            WY_B = CT[0:bl, OL["wyr"] : OL["wyr"] + DY]

            ONESC = pp.tile([128, 1], F32, tag="ones")
            nc.gpsimd.memset(ONESC[:], 1.0)
            # preload ACT Exp/Ln tables so the late softmax doesn't stall
            DUM = sm.tile([1, 1], F32, tag="dum")
            nc.scalar.activation(DUM[:], ONESC[0:1, 0:1], ACTF.Exp)
            nc.scalar.activation(DUM[:], DUM[:], ACTF.Ln)

            CNT = pp.tile([128, bl], F32, tag="cnt")  # per-row sum(NEG) partials
            SEP = pp.tile([128, bl], F32, tag="sep")  # per-row E*we chunk partials
            MX8 = pp.tile([128, 8 * bl], F32, tag="mx8")  # per-row top8 of m
            MI8 = pp.tile([128, 8 * bl], U32, tag="mi8")

            # ---- small batched inputs ----
            GT = pp.tile([128, bl * N], F32, tag="gt")  # g_edge (premasked), [i,(g j)]
            nc.gpsimd.dma_start(
                GT[:].rearrange("i (g j) -> i g j", g=bl),
                ge[:].rearrange("g i j -> i g j"),
            )
            XT = pp.tile([128, bl * (DX + 1)], F32, tag="xt")  # X, [n, (g c)]
            nc.scalar.dma_start(
                XT[:].rearrange("n (g c) -> n g c", g=bl),
                X_in[:].rearrange("g n c -> n g c"),
            )
            GN = pp.tile([128, bl], F32, tag="gn")  # g_node, [n, g]
            nc.scalar.dma_start(GN[:], gn[:].rearrange("g n -> n g"))
            Y32 = pp.tile([bl, DY], F32, tag="y32")
            nc.scalar.dma_start(Y32[:], yy[:])

            # ---- phase 2: X-side stats ----
            XT3 = XT[:].rearrange("n (g c) -> n g c", g=bl)
            X8 = XT3[:, :, DX]  # [128, bl] masked-node indicator
            SXS = sm.tile([128, bl * (DX + 1)], F32, tag="sxs")
            nc.vector.tensor_tensor(out=SXS[:], in0=XT[:], in1=WXR, op=ALU.mult)
            SXP = sm.tile([128, bl], F32, tag="sxp")
            nc.vector.tensor_reduce(
                out=SXP[:], in_=SXS[:].rearrange("n (g c) -> n g c", g=bl),
                axis=AX.X, op=ALU.add,
            )
            NEGN = sm.tile([128, bl], F32, tag="negn")
            nc.scalar.activation(NEGN[:], X8, ACTF.Copy, bias=-BIG, scale=BIG)
            MN = sm.tile([128, bl], F32, tag="mn")
            nc.vector.tensor_tensor(out=MN[:], in0=GN[:], in1=NEGN[:], op=ALU.add)
            MNT_P = psp.tile([bl, 128], F32, tag="tp")
            nc.tensor.transpose(out=MNT_P[:], in_=MN[:], identity=IDENT)
            MNT = sm.tile([bl, 128], F32, tag="mnt")
            nc.vector.tensor_copy(out=MNT[:], in_=MNT_P[:])
            NMX = sm.tile([bl, 8], F32, tag="nmx")
            NIX = sm.tile([bl, 8], U32, tag="nix")
            nc.vector.max_with_indices(out_max=NMX[:], out_indices=NIX[:], in_=MNT[:])
            UNF = sm.tile([bl, 1], F32, tag="unf")
            nc.vector.tensor_copy(out=UNF[:], in_=NIX[:, 0:1])
            nc.vector.tensor_scalar(
                out=UNF[:], in0=UNF[:], scalar1=float(N - 1), scalar2=0.0,
                op0=ALU.min, op1=ALU.max,
            )

            P4 = psp.tile([bl, 4], F32, tag="p4")
            P_CNTN, P_CNTE, P_SE, P_SX = (P4[:, i : i + 1] for i in range(4))
            nc.tensor.matmul(out=P_CNTN, lhsT=X8, rhs=ONESC[:], start=True, stop=True)
            nc.tensor.matmul(out=P_SX, lhsT=SXP[:], rhs=ONESC[:], start=True, stop=True)
            CNTN = sm.tile([bl, 1], F32, tag="cntn")
            nc.vector.tensor_copy(out=CNTN[:], in_=P_CNTN)
            SX = sm.tile([bl, 1], F32, tag="sx")
            nc.vector.tensor_copy(out=SX[:], in_=P_SX)
            GATE_N = sm.tile([bl, 1], F32, tag="gate_n")
            nc.vector.tensor_scalar(
                out=GATE_N[:], in0=CNTN[:], scalar1=1.0, scalar2=None, op0=ALU.min
            )
            OFF_N = sm.tile([bl, 1], F32, tag="off_n")
            nc.vector.tensor_scalar(
                out=OFF_N[:], in0=GATE_N[:], scalar1=-OOB, scalar2=OOB,
                op0=ALU.mult, op1=ALU.add,
            )
            GNF = sm.tile([bl, 1], F32, tag="gnf")
            nc.vector.scalar_tensor_tensor(
                out=GNF[:], in0=IOTAC, scalar=float(N), in1=OFF_N[:],
                op0=ALU.mult, op1=ALU.add,
            )
            FN = sm.tile([bl, 1], F32, tag="fn")
            nc.vector.tensor_tensor(out=FN[:], in0=GNF[:], in1=UNF[:], op=ALU.add)
            FNI = sm.tile([bl, 1], I32, tag="fni")
            nc.vector.tensor_copy(out=FNI[:], in_=FN[:])

            LXG = sm.tile([bl, DX], F32, tag="lxg")
            nc.gpsimd.memset(LXG[:], 0.0)
            GXG = sm.tile([bl, DX], F32, tag="gxg")
            nc.gpsimd.memset(GXG[:], 0.0)
            nc.gpsimd.indirect_dma_start(
                out=LXG[:], out_offset=None,
                in_=lx[:].rearrange("g n c -> (g n) c"),
                in_offset=bass.IndirectOffsetOnAxis(ap=FNI[:, 0:1], axis=0),
                bounds_check=bl * N - 1, oob_is_err=False,
            )
            nc.gpsimd.indirect_dma_start(
                out=GXG[:], out_offset=None,
                in_=gcx[:].rearrange("g n c -> (g n) c"),
                in_offset=bass.IndirectOffsetOnAxis(ap=FNI[:, 0:1], axis=0),
                bounds_check=bl * N - 1, oob_is_err=False,
            )

            # ---- phase 1: stream E, per-graph row stats ----
            bulk_writes = []
            GT3 = GT[:].rearrange("i (g j) -> i g j", g=bl)
            for g0 in range(0, bl, group):
                T = epool.tile([128, group * EW], F32, tag="etile")
                T3 = T[:].rearrange("p (g f) -> p g f", g=group)
                eng_in = [nc.sync, nc.scalar, nc.sync, nc.scalar,
                          nc.sync, nc.sync, nc.scalar, nc.sync][(g0 // group) % 8]
                eng_in.dma_start(
                    T3, E_in[g0 : g0 + group].rearrange("g i j c -> i g (j c)")
                )
                eng_out = [nc.gpsimd, nc.gpsimd, nc.gpsimd, nc.gpsimd,
                           nc.gpsimd, nc.sync, nc.scalar, nc.sync][(g0 // group) % 8]
                w = eng_out.dma_start(
                    E_out[g0 : g0 + group].rearrange("g i j c -> i g (j c)"), T3
                )
                bulk_writes.append(w)
                # grouped stats: one ACT + three DVE ops per 4-graph group
                E5G = T3.rearrange("p g (j c) -> p g j c", c=DE + 1)[:, :, :, DE]
                NEG4 = spool.tile([128, group * N], F32, tag="neg4")
                nc.scalar.activation(
                    NEG4[:].rearrange("p (g j) -> p g j", g=group), E5G,
                    ACTF.Copy, bias=-BIG, scale=BIG,
                )
                nc.vector.tensor_reduce(
                    out=CNT[:, g0 : g0 + group],
                    in_=NEG4[:].rearrange("p (g j) -> p g j", g=group),
                    axis=AX.X, op=ALU.add,
                )
                M4 = spool.tile([128, group * N], F32, tag="m4")
                nc.vector.tensor_tensor(
                    out=M4[:], in0=GT3[:, g0 : g0 + group].rearrange("p g j -> p (g j)"),
                    in1=NEG4[:], op=ALU.add,
                )
                PS6G = psp6.tile([128, group * NCHUNK], F32, tag="ps6g")
                for k in range(group):
                    g = g0 + k
                    nc.vector.max_with_indices(
                        out_max=MX8[:, 8 * g : 8 * g + 8],
                        out_indices=MI8[:, 8 * g : 8 * g + 8],
                        in_=M4[:, k * N : (k + 1) * N],
                    )
                    for c in range(NCHUNK):
                        nc.tensor.matmul(
                            out=PS6G[:, k * NCHUNK + c : k * NCHUNK + c + 1],
                            lhsT=T3[:, k][:, c * N : (c + 1) * N],
                            rhs=ONESC[:],
                            start=True,
                            stop=True,
                        )
                SCR6 = spool.tile([128, group * NCHUNK], F32, tag="scr6")
                nc.vector.tensor_tensor(
                    out=SCR6[:], in0=PS6G[:], in1=W2G, op=ALU.mult
                )
                nc.vector.tensor_reduce(
                    out=SEP[:, g0 : g0 + group],
                    in_=SCR6[:].rearrange("p (g k) -> p g k", g=group),
                    axis=AX.X, op=ALU.add,
                )

            # X_out bulk write (from XT, unmodified)
            xw = nc.gpsimd.dma_start(
                X_out[:].rearrange("g n c -> n g c"),
                XT[:].rearrange("n (g c) -> n g c", g=bl),
            )

            # ---- phase 3: edge argmax (u, v) ----
            RMAX = sm.tile([128, bl], F32, tag="rmax")
            nc.vector.tensor_copy(
                out=RMAX[:], in_=MX8[:].rearrange("p (g e) -> p g e", e=8)[:, :, 0]
            )
            RIDX = sm.tile([128, bl], F32, tag="ridx")
            nc.vector.tensor_copy(
                out=RIDX[:], in_=MI8[:].rearrange("p (g e) -> p g e", e=8)[:, :, 0]
            )
            RMAXT_P = psp.tile([bl, 128], F32, tag="tp2")
            nc.tensor.transpose(out=RMAXT_P[:], in_=RMAX[:], identity=IDENT)
            RMAXT = sm.tile([bl, 128], F32, tag="rmaxt")
            nc.vector.tensor_copy(out=RMAXT[:], in_=RMAXT_P[:])
            RIDXT_P = psp.tile([bl, 128], F32, tag="tp3")
            nc.tensor.transpose(out=RIDXT_P[:], in_=RIDX[:], identity=IDENT)
            RIDXT = sm.tile([bl, 128], F32, tag="ridxt")
            nc.vector.tensor_copy(out=RIDXT[:], in_=RIDXT_P[:])
            EMX = sm.tile([bl, 8], F32, tag="emx")
            EIX = sm.tile([bl, 8], U32, tag="eix")
            nc.vector.max_with_indices(out_max=EMX[:], out_indices=EIX[:], in_=RMAXT[:])
            UEF = sm.tile([bl, 1], F32, tag="uef")
            nc.vector.tensor_copy(out=UEF[:], in_=EIX[:, 0:1])
            nc.vector.tensor_scalar(
                out=UEF[:], in0=UEF[:], scalar1=float(N - 1), scalar2=0.0,
                op0=ALU.min, op1=ALU.max,
            )
            EQM = sm.tile([bl, 128], F32, tag="eqm")
            nc.vector.tensor_scalar(
                out=EQM[:], in0=RMAXT[:], scalar1=EMX[:, 0:1], scalar2=None,
                op0=ALU.is_equal,
            )
            VSCR = sm.tile([bl, 128], F32, tag="vscr")
            VF = sm.tile([bl, 1], F32, tag="vf")
            nc.vector.tensor_tensor(out=VSCR[:], in0=EQM[:], in1=RIDXT[:], op=ALU.mult)
            nc.vector.tensor_reduce(out=VF[:], in_=VSCR[:], axis=AX.X, op=ALU.add)
            nc.vector.tensor_scalar(
                out=VF[:], in0=VF[:], scalar1=float(N - 1), scalar2=0.0,
                op0=ALU.min, op1=ALU.max,
            )

            # ---- phase 3: per-graph totals via matmul (partition = graph) ----
            nc.tensor.matmul(out=P_CNTE, lhsT=CNT[:], rhs=ONESC[:], start=True, stop=True)
            nc.tensor.matmul(out=P_SE, lhsT=SEP[:], rhs=ONESC[:], start=True, stop=True)
            # masked_e = N^2/2 + sum(NEG)/2^31  (sum E5 == 2*masked_e, diag 0)
            ME = sm.tile([bl, 1], F32, tag="me")
            nc.vector.tensor_scalar(
                out=ME[:], in0=P_CNTE, scalar1=float(2.0 ** -31),
                scalar2=float(N * N / 2), op0=ALU.mult, op1=ALU.add,
            )
            SE = sm.tile([bl, 1], F32, tag="se")
            nc.vector.tensor_copy(out=SE[:], in_=P_SE)

            # Sy
            YS = sm.tile([bl, DY], F32, tag="ys")
            SY = sm.tile([bl, 1], F32, tag="sy")
            nc.vector.tensor_tensor(out=YS[:], in0=Y32[:], in1=WY_B, op=ALU.mult)
            nc.vector.tensor_reduce(out=SY[:], in_=YS[:], axis=AX.X, op=ALU.add)

            # temp = 0.5 - (cntn + masked_e)*(0.5/8256); base = SX/N + SE/N^2 + SY
            NUMEL = N + N * (N - 1) / 2.0
            T1 = sm.tile([bl, 1], F32, tag="t1")
            nc.vector.tensor_tensor(out=T1[:], in0=CNTN[:], in1=ME[:], op=ALU.add)
            TEMP = sm.tile([bl, 1], F32, tag="temp")
            nc.vector.tensor_scalar(
                out=TEMP[:], in0=T1[:], scalar1=-0.5 / NUMEL, scalar2=0.5,
                op0=ALU.mult, op1=ALU.add,
            )
            B1 = sm.tile([bl, 1], F32, tag="b1")
            nc.vector.tensor_scalar(
                out=B1[:], in0=SX[:], scalar1=1.0 / N, scalar2=None, op0=ALU.mult
            )
            B2 = sm.tile([bl, 1], F32, tag="b2")
            nc.vector.scalar_tensor_tensor(
                out=B2[:], in0=SE[:], scalar=1.0 / (N * N), in1=SY[:],
                op0=ALU.mult, op1=ALU.add,
            )
            BASE = sm.tile([bl, 1], F32, tag="base")
            nc.vector.tensor_tensor(out=BASE[:], in0=B1[:], in1=B2[:], op=ALU.add)

            # ---- gates and flat indices ----
            GATE_E = sm.tile([bl, 1], F32, tag="gate_e")
            nc.vector.tensor_scalar(
                out=GATE_E[:], in0=ME[:], scalar1=1.0, scalar2=None, op0=ALU.min
            )
            OFF_E = sm.tile([bl, 1], F32, tag="off_e")  # (1-gate)*OOB
            nc.vector.tensor_scalar(
                out=OFF_E[:], in0=GATE_E[:], scalar1=-OOB, scalar2=OOB,
                op0=ALU.mult, op1=ALU.add,
            )
            GOF = sm.tile([bl, 1], F32, tag="gof")  # g*N^2 + (1-gate)*OOB
            nc.vector.scalar_tensor_tensor(
                out=GOF[:], in0=IOTAC, scalar=float(N * N), in1=OFF_E[:],
                op0=ALU.mult, op1=ALU.add,
            )
            FE = sm.tile([bl, 1], F32, tag="fe")  # g*N^2 + u*N + v (+oob)
            nc.vector.scalar_tensor_tensor(
                out=FE[:], in0=UEF[:], scalar=float(N), in1=GOF[:],
                op0=ALU.mult, op1=ALU.add,
            )
            nc.vector.tensor_tensor(out=FE[:], in0=FE[:], in1=VF[:], op=ALU.add)
            FS = sm.tile([bl, 1], F32, tag="fs")  # g*N^2 + v*N + u (+oob)
            nc.vector.scalar_tensor_tensor(
                out=FS[:], in0=VF[:], scalar=float(N), in1=GOF[:],
                op0=ALU.mult, op1=ALU.add,
            )
            nc.vector.tensor_tensor(out=FS[:], in0=FS[:], in1=UEF[:], op=ALU.add)
            FEI = sm.tile([bl, 1], I32, tag="fei")
            nc.vector.tensor_copy(out=FEI[:], in_=FE[:])
            FSI = sm.tile([bl, 1], I32, tag="fsi")
            nc.vector.tensor_copy(out=FSI[:], in_=FS[:])

            # ---- gathers at the selected positions ----
            LEG = sm.tile([bl, DE], F32, tag="leg")
            nc.gpsimd.memset(LEG[:], 0.0)
            GEG = sm.tile([bl, DE], F32, tag="geg")
            nc.gpsimd.memset(GEG[:], 0.0)
            nc.gpsimd.indirect_dma_start(
                out=LEG[:], out_offset=None,
                in_=le[:].rearrange("g u v c -> (g u v) c"),
                in_offset=bass.IndirectOffsetOnAxis(ap=FEI[:, 0:1], axis=0),
                bounds_check=bl * N * N - 1, oob_is_err=False,
            )
            nc.gpsimd.indirect_dma_start(
                out=GEG[:], out_offset=None,
                in_=gce[:].rearrange("g u v c -> (g u v) c"),
                in_offset=bass.IndirectOffsetOnAxis(ap=FEI[:, 0:1], axis=0),
                bounds_check=bl * N * N - 1, oob_is_err=False,
            )

            def softmax_scores(tag, LOG, GCAT, W_ROW, nclass, dwscale):
                # d_i = temp*(base + dwscale*(w_i - w_last)); returns scores
                DD = sm.tile([bl, nclass], F32, tag=tag + "_dd")
                nc.vector.tensor_scalar(
                    out=DD[:], in0=W_ROW[:, 0:nclass],
                    scalar1=W_ROW[:, nclass : nclass + 1], scalar2=dwscale,
                    op0=ALU.subtract, op1=ALU.mult,
                )
                nc.vector.tensor_scalar(
                    out=DD[:], in0=DD[:], scalar1=BASE[:, 0:1], scalar2=TEMP[:, 0:1],
                    op0=ALU.add, op1=ALU.mult,
                )
                ED = sm.tile([bl, nclass], F32, tag=tag + "_ed")
                nc.scalar.activation(ED[:], DD[:], ACTF.Exp)
                MXL = sm.tile([bl, 1], F32, tag=tag + "_mxl")
                nc.vector.tensor_reduce(out=MXL[:], in_=LOG[:], axis=AX.X, op=ALU.max)
                NMXL = sm.tile([bl, 1], F32, tag=tag + "_nmxl")
                nc.vector.tensor_scalar(
                    out=NMXL[:], in0=MXL[:], scalar1=-1.0, scalar2=None, op0=ALU.mult
                )
                EXL = sm.tile([bl, nclass], F32, tag=tag + "_exl")
                nc.scalar.activation(EXL[:], LOG[:], ACTF.Exp, bias=NMXL[:, 0:1])
                SML = sm.tile([bl, 1], F32, tag=tag + "_sml")
                nc.vector.tensor_reduce(out=SML[:], in_=EXL[:], axis=AX.X, op=ALU.add)
                RSL = sm.tile([bl, 1], F32, tag=tag + "_rsl")
                nc.vector.reciprocal(out=RSL[:], in_=SML[:])
                P = sm.tile([bl, nclass], F32, tag=tag + "_p")
                nc.vector.tensor_scalar(
                    out=P[:], in0=EXL[:], scalar1=RSL[:, 0:1], scalar2=None,
                    op0=ALU.mult,
                )
                GP = sm.tile([bl, nclass], F32, tag=tag + "_gp")
                nc.vector.tensor_tensor(out=GP[:], in0=P[:], in1=ED[:], op=ALU.mult)
                nc.vector.tensor_scalar(
                    out=GP[:], in0=GP[:], scalar1=1e-30, scalar2=None, op0=ALU.add
                )
                LGP = sm.tile([bl, nclass], F32, tag=tag + "_lgp")
                nc.scalar.activation(LGP[:], GP[:], ACTF.Ln)
                SCO = sm.tile([bl, nclass], F32, tag=tag + "_sco")
                nc.vector.tensor_tensor(out=SCO[:], in0=LGP[:], in1=GCAT[:], op=ALU.add)
                return SCO

            # edge prediction
            SCOE = softmax_scores("e", LEG, GEG, WE_B, DE, 2.0 / (N * N))
            SC8E = sm.tile([bl, 8], F32, tag="sc8e")
            nc.gpsimd.memset(SC8E[:], -1e30)
            nc.vector.tensor_copy(out=SC8E[:, 0:DE], in_=SCOE[:])
            PEX = sm.tile([bl, 8], F32, tag="pex")
            PEI = sm.tile([bl, 8], U32, tag="pei")
            nc.vector.max_with_indices(out_max=PEX[:], out_indices=PEI[:], in_=SC8E[:])
            PEF = sm.tile([bl, 1], F32, tag="pef")
            nc.vector.tensor_copy(out=PEF[:], in_=PEI[:, 0:1])
            OH6 = sm.tile([bl, DE + 1], F32, tag="oh6")
            nc.vector.tensor_scalar(
                out=OH6[:], in0=IOTA6, scalar1=PEF[:, 0:1], scalar2=None,
                op0=ALU.is_equal,
            )

            # node prediction
            SCOX = softmax_scores("x", LXG, GXG, WX_B, DX, 1.0 / N)
            PXX = sm.tile([bl, 8], F32, tag="pxx")
            PXI = sm.tile([bl, 8], U32, tag="pxi")
            nc.vector.max_with_indices(out_max=PXX[:], out_indices=PXI[:], in_=SCOX[:])
            PXF = sm.tile([bl, 1], F32, tag="pxf")
            nc.vector.tensor_copy(out=PXF[:], in_=PXI[:, 0:1])
            OH9 = sm.tile([bl, DX + 1], F32, tag="oh9")
            nc.vector.tensor_scalar(
                out=OH9[:], in0=IOTA9, scalar1=PXF[:, 0:1], scalar2=None,
                op0=ALU.is_equal,
            )

            # ---- scatters (ordered after bulk writes) ----
            s1 = nc.gpsimd.indirect_dma_start(
                out=E_out[:].rearrange("g u v c -> (g u v) c"),
                out_offset=bass.IndirectOffsetOnAxis(ap=FEI[:, 0:1], axis=0),
                in_=OH6[:], in_offset=None,
                bounds_check=bl * N * N - 1, oob_is_err=False,
            )
            s2 = nc.gpsimd.indirect_dma_start(
                out=E_out[:].rearrange("g u v c -> (g u v) c"),
                out_offset=bass.IndirectOffsetOnAxis(ap=FSI[:, 0:1], axis=0),
                in_=OH6[:], in_offset=None,
                bounds_check=bl * N * N - 1, oob_is_err=False,
            )
            s3 = nc.gpsimd.indirect_dma_start(
                out=X_out[:].rearrange("g n c -> (g n) c"),
                out_offset=bass.IndirectOffsetOnAxis(ap=FNI[:, 0:1], axis=0),
                in_=OH9[:], in_offset=None,
                bounds_check=bl * N - 1, oob_is_err=False,
            )
            for w in bulk_writes:
                add_dep_helper(s1.ins, w.ins, sync=True, reason="scatter after bulk E")
                add_dep_helper(s2.ins, w.ins, sync=True, reason="scatter after bulk E")
            add_dep_helper(s3.ins, xw.ins, sync=True, reason="scatter after bulk X")

    # Force every activation onto one table set (6: natural_log_exp_and_others,
    # which holds Copy/Identity/Exp/Ln) so only one LoadActFuncSet is emitted
    # instead of thrashing 0<->5 on every Copy/Exp/Ln transition.
    import concourse.bacc as _bacc_mod
    from concourse.hw_specs import get_activation_tables as _gat

    def _one_set(arch):
        t = _gat(arch)
        keys = list(t.keys())
        want = {ACTF.Copy, ACTF.Identity, ACTF.Exp, ACTF.Ln}
        full = [k for k in keys if want <= t[k]]
        assert full, f"no act set holds {want}; sets: {keys}"
        keep = full[0]
        return {k: (v if k == keep else set()) for k, v in t.items()}

    _orig = _bacc_mod.get_activation_tables
    _bacc_mod.get_activation_tables = _one_set
    try:
        nc.compile()
    finally:
        _bacc_mod.get_activation_tables = _orig
    return nc


_NC_CACHE = {}


def _get_nc(bl=BL, group=4):
    key = (bl, group)
    if key not in _NC_CACHE:
        _NC_CACHE[key] = build_nc(bl, group)
    return _NC_CACHE[key]


def make_in_maps(inputs, bl=BL, ncores=NCORES):
    f = lambda a: np.ascontiguousarray(np.asarray(a, dtype=np.float32))
    consts = build_consts(f(inputs["we"]), f(inputs["wx"]), f(inputs["wy"]), bl=bl)
    gem = premask_g_edge(inputs["g_edge"])
    in_maps = []
    for i in range(ncores):
        s = slice(i * bl, (i + 1) * bl)
        in_maps.append(
            {
                "E": f(inputs["E"][s]),
                "g_edge": np.ascontiguousarray(gem[s]),
                "X": f(inputs["X"][s]),
                "g_node": f(inputs["g_node"][s]),
                "logits_x": f(inputs["logits_x"][s]),
                "g_cat_x": f(inputs["g_cat_x"][s]),
                "logits_e": f(inputs["logits_e"][s]),
                "g_cat_e": f(inputs["g_cat_e"][s]),
                "y": f(inputs["y"][s]),
                "consts": consts,
            }
        )
    return in_maps


def kernel(**inputs):
    nc = _get_nc()
    in_maps = make_in_maps(inputs)
    res = run_bass_kernel_spmd(nc, in_maps, core_ids=list(range(NCORES)))
    X_out = np.concatenate([res.results[i]["X_out"] for i in range(NCORES)], axis=0)
    E_out = np.concatenate([res.results[i]["E_out"] for i in range(NCORES)], axis=0)
    return X_out, E_out
